# revision 1
# baseline (speedup 1.0000x reference)
"""Attention pooling kernel for Trainium2 (8 NeuronCores).

Computes: scores = E @ q; w = softmax(scores); out = w @ E
for E [N=2097152, 64] fp32, q [64] fp32.

Strategy (per core, N/8 = 262144 rows):
  - Host packs the core's row-shard into a "2-row-packed transposed" layout
    Epack [128, C=131072]: partition k = p*64 + d holds E[2n + p, d] at
    column n.  DMA is then contiguous per partition.
  - Scores via one TensorE matmul per 512-column chunk with a constant
    stationary operand qmat [128, 128], qmat[k, m] = q[k % 64] if
    (k // 64 == m // 64) else 0.  out[m, n] = s(2n + m//64): scores land
    REPLICATED across the 64 partitions of each parity half -> exp can use
    all 128 ACT lanes and the weights are already broadcast for the
    weighted-sum multiply.
  - ACT: w = exp(scores - C) PSUM->SBUF, fused accum_out gives per-chunk
    sumexp partials.
  - DVE: fused scalar_tensor_tensor (out = Epack * w, accum_out = per
    partition sum) accumulates the weighted sum: partition k = (p, d) gets
    sum_n E[2n+p, d] * w(2n+p).
  - Host: out[d] = (acc[d] + acc[64+d]) / (se[0] + se[64]), summed over
    cores.  The shift C (computed from q alone) cancels.
"""

import sys

sys.path.insert(0, "/opt/trn_rl_repo")

import numpy as np

N_TOTAL = 2097152
D = 64
N_CORES = 8
N_PER_CORE = N_TOTAL // N_CORES          # 262144
COLS_PER_CORE = N_PER_CORE // 2          # 131072 packed columns
MM_N = 512                               # matmul free dim (one PSUM bank)
DMA_COLS = 8192                          # columns per DMA tile
EXP_COLS = 1024                          # columns per exp op (2 PSUM banks)
MM_DT_NAME = "float32r"                  # scores matmul dtype; "float32" fallback
HWDGE_LANES = 1                          # DMA completion-sem lanes

_compiled = {}


def _build_nc(n_cols, dma_cols, mm_dt_name):
    import concourse.bacc as bacc
    import concourse.bass as bass
    import concourse.mybir as mybir
    import concourse.tile as tile

    fp32 = mybir.dt.float32
    bf16 = mybir.dt.bfloat16
    mm_dt = getattr(mybir.dt, mm_dt_name)

    nc = bacc.Bacc()
    ep_dram = nc.declare_dram_parameter("epack", [128, n_cols], mm_dt, isOutput=False)
    qmat_dram = nc.declare_dram_parameter("qmat", [128, 128], mm_dt, isOutput=False)
    cshift_dram = nc.declare_dram_parameter("cshift", [128, 1], fp32, isOutput=False)
    out_dram = nc.declare_dram_parameter("out", [128, 2], fp32, isOutput=True)

    n_tiles = n_cols // dma_cols

    with tile.TileContext(nc) as tc:
        with (
            tc.tile_pool(name="const", bufs=1) as const_pool,
            tc.tile_pool(name="ep", bufs=3) as ep_pool,
            tc.tile_pool(name="w", bufs=2) as w_pool,
            tc.tile_pool(name="junk", bufs=1) as junk_pool,
            tc.tile_pool(name="acc", bufs=1) as acc_pool,
            tc.tile_pool(name="se", bufs=4) as se_pool,
            tc.tile_pool(name="aw", bufs=3) as aw_pool,
            tc.tile_pool(name="psum", bufs=4, space=bass.MemorySpace.PSUM) as psum_pool,
        ):
            qmat = const_pool.tile([128, 128], mm_dt, tag="qmat")
            cshift = const_pool.tile([128, 1], fp32, tag="cshift")
            nc.sync.dma_start(qmat[:], qmat_dram[:])
            nc.sync.dma_start(cshift[:], cshift_dram[:])

            master_w = acc_pool.tile([128, 1], fp32, tag="master_w")
            master_se = acc_pool.tile([128, 1], fp32, tag="master_se")
            tmp_se = acc_pool.tile([128, 1], fp32, tag="tmp_se")

            groups = dma_cols // EXP_COLS
            for t in range(n_tiles):
                ep = ep_pool.tile([128, dma_cols], mm_dt, tag="ep")
                nc.sync.dma_start(ep[:], ep_dram[:, t * dma_cols:(t + 1) * dma_cols])

                w_sb = w_pool.tile([128, dma_cols], fp32, tag="w")
                accse = se_pool.tile([128, groups], fp32, tag="accse")
                for g in range(groups):
                    lo = g * EXP_COLS
                    ps = psum_pool.tile([128, EXP_COLS], fp32, tag="ps")
                    for k in range(EXP_COLS // MM_N):
                        nc.tensor.matmul(
                            ps[:, k * MM_N:(k + 1) * MM_N],
                            qmat[:],
                            ep[:, lo + k * MM_N:lo + (k + 1) * MM_N],
                            start=True,
                            stop=True,
                        )
                    # w = exp(scores - C); accum gives per-group sumexp
                    nc.scalar.activation(
                        w_sb[:, lo:lo + EXP_COLS],
                        ps[:],
                        mybir.ActivationFunctionType.Exp,
                        bias=cshift[:, 0:1],
                        scale=1.0,
                        accum_out=accse[:, g:g + 1],
                    )
                junk = junk_pool.tile([128, dma_cols], bf16, tag="junk")
                accw = aw_pool.tile([128, 1], fp32, tag="accw")
                nc.vector.scalar_tensor_tensor(
                    junk[:],
                    ep[:].bitcast(fp32),
                    1.0,
                    w_sb[:],
                    op0=mybir.AluOpType.mult,
                    op1=mybir.AluOpType.mult,
                    accum_out=accw[:],
                )
                if t == 0:
                    nc.vector.tensor_copy(master_w[:], accw[:])
                else:
                    nc.vector.tensor_add(master_w[:], master_w[:], accw[:])
                # fold sumexp partials (light DVE ops)
                nc.vector.tensor_reduce(
                    tmp_se[:], accse[:], axis=mybir.AxisListType.X,
                    op=mybir.AluOpType.add,
                )
                if t == 0:
                    nc.vector.tensor_copy(master_se[:], tmp_se[:])
                else:
                    nc.vector.tensor_add(master_se[:], master_se[:], tmp_se[:])

            res = acc_pool.tile([128, 2], fp32, tag="res")
            nc.vector.tensor_copy(res[:, 0:1], master_w[:])
            nc.vector.tensor_copy(res[:, 1:2], master_se[:])
            nc.sync.dma_start(out_dram[:], res[:])

    nc.compile()
    return nc


def _pack_core(e_core):
    # [Nc, 64] -> [n, p, d] -> [(p, d), n]
    nc_rows = e_core.shape[0]
    return np.ascontiguousarray(
        e_core.reshape(nc_rows // 2, 2, D).transpose(1, 2, 0).reshape(128, nc_rows // 2)
    )


def kernel(embeddings, query):
    from concourse.bass_utils import run_bass_kernel_spmd

    embeddings = np.asarray(embeddings, dtype=np.float32)
    query = np.asarray(query, dtype=np.float32)

    key = (COLS_PER_CORE, DMA_COLS, MM_DT_NAME)
    if key not in _compiled:
        _compiled[key] = _build_nc(*key)
    nc = _compiled[key]

    # constant shift for exp stability; cancels in the final division
    c_shift = float(6.0 * np.linalg.norm(query))

    qmat = np.zeros((128, 128), dtype=np.float32)
    qmat[0:64, 0:64] = query[:, None]      # qmat[k, m] = q[k] for m in first half
    qmat[64:128, 64:128] = query[:, None]
    cshift = np.full((128, 1), -c_shift, dtype=np.float32)

    in_maps = []
    for c in range(N_CORES):
        e_core = embeddings[c * N_PER_CORE:(c + 1) * N_PER_CORE]
        in_maps.append({
            "epack": _pack_core(e_core),
            "qmat": qmat,
            "cshift": cshift,
        })

    res = None
    for attempt in range(3):
        try:
            res = run_bass_kernel_spmd(nc, in_maps, list(range(N_CORES)))
            break
        except Exception:
            if attempt == 2:
                raise

    wsum = np.zeros(D, dtype=np.float64)
    sumexp = 0.0
    for r in res.results:
        out = r["out"].astype(np.float64)
        wsum += out[0:64, 0] + out[64:128, 0]
        sumexp += out[0, 1] + out[64, 1]
    return (wsum / sumexp).astype(np.float32)



# revision 6
# speedup vs baseline: 1.3637x; 1.3637x over previous
"""Attention pooling kernel for Trainium2 (8 NeuronCores) — bf16 pipeline.

Computes: scores = E @ q; w = softmax(scores); out = w @ E
for E [N=2097152, 64] fp32, q [64] fp32.

Strategy (per core, N/8 = 262144 rows), all heavy traffic in bf16:
  - Host casts the core's row-shard to bf16 and packs it as
    ep [128, 131072]: with n = 16*f + j (j in [0,16)) and d = 8*g + e
    (e in [0,8)), partition k = 8*j + e holds E[16f+j, 8g+e] at column
    ch*8192 + g*1024 + fl  (f = ch*1024 + fl; 16 chunks of 8192 cols).
  - Scores: 8 chained matmuls per 512-col half (stationary Qg [128,128],
    Qg[8j+e, 8j'+e'] = q[8g+e]*(j==j')) accumulate in PSUM ->
    scores s(16f+j) land replicated over the 8 e'-slots of parity j.
    exp therefore only touches [128, F] per chunk — 8x less ACT work
    than a 2-parity layout.
  - ACT: w = exp(scores - C) -> f16, fused accum_out gives sumexp
    partials (C computed from q alone; cancels in the final division).
  - DVE: per chunk 8 tensor_tensor bf16 multiplies (2x perf mode):
    prod_g = ep_g * w.  The f-reduction of prod_g is split:
      g=0..3 -> ACT Copy with accum_out over 4-chunk spans
      g=4..7 -> PE identity-stationary matmuls accumulating into 4
                persistent PSUM banks (emitted one chunk late so PE
                never stalls on the exp->mult chain).
  - Host: out[d] = sum_cores sum_j acc[(j,e),g] / sum exp, d = 8g+e.
"""

import sys

sys.path.insert(0, "/opt/trn_rl_repo")

import numpy as np

N_TOTAL = 2097152
D = 64
N_CORES = 8
N_PER_CORE = N_TOTAL // N_CORES          # 262144
J = 16                                   # parities (n mod 16)
G = 8                                    # dim groups of 8
F_TOT = N_PER_CORE // J                  # 16384 f-columns per core
F = 1024                                 # f-columns per chunk
N_CHUNKS = F_TOT // F                    # 16
CH_COLS = G * F                          # 8192 packed cols per chunk
COLS = F_TOT * G                         # 131072 packed cols per core
SPAN = 4                                 # chunks per ACT reduce span
G_ACT = 4                                # groups 0..3 reduced on ACT
MM_N = 512                               # matmul free dim (one PSUM bank)

_compiled = {}


def _build_nc():
    import concourse.bacc as bacc
    import concourse.bass as bass
    import concourse.mybir as mybir
    import concourse.tile as tile

    fp32 = mybir.dt.float32
    f16 = mybir.dt.float16

    nc = bacc.Bacc()
    ep_dram = nc.declare_dram_parameter("epack", [128, COLS], f16, isOutput=False)
    qm_dram = nc.declare_dram_parameter("qmats", [128, 128 * G], f16, isOutput=False)
    id_dram = nc.declare_dram_parameter("ident", [128, 128], f16, isOutput=False)
    cs_dram = nc.declare_dram_parameter("cshift", [128, 1], fp32, isOutput=False)
    out_dram = nc.declare_dram_parameter("out", [128, 9], fp32, isOutput=True)

    n_halves = F // MM_N
    G_PE = G - G_ACT

    with tile.TileContext(nc) as tc:
        with (
            tc.tile_pool(name="const", bufs=1) as const_pool,
            tc.tile_pool(name="ep", bufs=5) as ep_pool,
            tc.tile_pool(name="w", bufs=3) as w_pool,
            tc.tile_pool(name="prodA", bufs=2) as prodA_pool,
            tc.tile_pool(name="prodS", bufs=2) as prodS_pool,
            tc.tile_pool(name="junk", bufs=1) as junk_pool,
            tc.tile_pool(name="sm", bufs=3) as sm_pool,
            tc.tile_pool(name="racc", bufs=2) as racc_pool,
            tc.tile_pool(name="master", bufs=1) as master_pool,
            tc.tile_pool(name="ps", bufs=2, space=bass.MemorySpace.PSUM) as ps_pool,
            tc.tile_pool(name="acc", bufs=1, space=bass.MemorySpace.PSUM) as acc_pool,
        ):
            qmats = const_pool.tile([128, 128 * G], f16, tag="qmats")
            ident = const_pool.tile([128, 128], f16, tag="ident")
            cshift = const_pool.tile([128, 1], fp32, tag="cshift")
            nc.sync.dma_start(qmats[:], qm_dram[:])
            nc.sync.dma_start(ident[:], id_dram[:])
            nc.sync.dma_start(cshift[:], cs_dram[:])

            master4 = master_pool.tile([128, G_ACT], fp32, tag="master4")
            master_se = master_pool.tile([128, 1], fp32, tag="master_se")
            accP = [
                acc_pool.tile([128, MM_N], fp32, tag=f"accP{gi}", name=f"accP{gi}")
                for gi in range(G_PE)
            ]

            pend = None          # (t, [prodS tiles]) awaiting PE reduce
            pend_act = None      # (prodA tile, racc tile) awaiting ACT reduce
            prodA = None

            def emit_pe_reduce(prev_t, tiles):
                for gi, pt in enumerate(tiles):
                    for h in range(n_halves):
                        nc.tensor.matmul(
                            accP[gi][:],
                            ident[:],
                            pt[:, h * MM_N:(h + 1) * MM_N],
                            start=(prev_t == 0 and h == 0),
                            stop=(prev_t == N_CHUNKS - 1 and h == n_halves - 1),
                        )

            for t in range(N_CHUNKS):
                ep = ep_pool.tile([128, CH_COLS], f16, tag="ep")
                nc.sync.dma_start(ep[:], ep_dram[:, t * CH_COLS:(t + 1) * CH_COLS])

                ps = ps_pool.tile([128, F], fp32, tag="ps")
                for h in range(n_halves):
                    lo = h * MM_N
                    for g in range(G):
                        nc.tensor.matmul(
                            ps[:, lo:lo + MM_N],
                            qmats[:, g * 128:(g + 1) * 128],
                            ep[:, g * F + lo:g * F + lo + MM_N],
                            start=(g == 0),
                            stop=(g == G - 1),
                        )
                # PE reduce of the previous chunk's prod tiles (one chunk
                # late so PE doesn't stall on the exp->mult chain)
                if pend is not None:
                    emit_pe_reduce(*pend)

                w = w_pool.tile([128, F], f16, tag="w")
                se = sm_pool.tile([128, 1], fp32, tag="se")
                nc.scalar.activation(
                    w[:],
                    ps[:],
                    mybir.ActivationFunctionType.Exp,
                    bias=cshift[:, 0:1],
                    scale=1.0,
                    accum_out=se[:],
                )
                # one ACT span-reduce op per chunk, for the previous span
                if pend_act is not None:
                    pa, racc = pend_act
                    g = t % SPAN
                    junk = junk_pool.tile([128, SPAN * F], f16, tag="junk")
                    nc.scalar.activation(
                        junk[:],
                        pa[:, g * SPAN * F:(g + 1) * SPAN * F],
                        mybir.ActivationFunctionType.Copy,
                        bias=0.0,
                        scale=1.0,
                        accum_out=racc[:, g:g + 1],
                    )
                    if g == G_ACT - 1:
                        if t == SPAN + G_ACT - 1:
                            nc.vector.tensor_copy(master4[:], racc[:])
                        else:
                            nc.vector.tensor_add(master4[:], master4[:], racc[:])
                        pend_act = None

                if t == 0:
                    nc.vector.tensor_copy(master_se[:], se[:])
                else:
                    nc.vector.tensor_add(master_se[:], master_se[:], se[:])

                if t % SPAN == 0:
                    prodA = prodA_pool.tile([128, G_ACT * SPAN * F], f16, tag="prodA")
                # PE-path mults first: the next chunk's PE reduce waits on them
                cur = []
                for g in range(G_ACT, G):
                    pS = prodS_pool.tile([128, F], f16, tag=f"prodS{g}")
                    nc.vector.tensor_mul(pS[:], ep[:, g * F:(g + 1) * F], w[:])
                    cur.append(pS)
                pend = (t, cur)
                for g in range(G_ACT):
                    dst = prodA[:, (g * SPAN + (t % SPAN)) * F:(g * SPAN + (t % SPAN) + 1) * F]
                    nc.vector.tensor_mul(dst, ep[:, g * F:(g + 1) * F], w[:])

                if t % SPAN == SPAN - 1:
                    racc = racc_pool.tile([128, G_ACT], fp32, tag="racc")
                    pend_act = (prodA, racc)

            # drain: last chunk's PE reduce, last ACT span, final combine
            emit_pe_reduce(*pend)
            pa, racc = pend_act
            junk = junk_pool.tile([128, SPAN * F], f16, tag="junk")
            for g in range(G_ACT):
                nc.scalar.activation(
                    junk[:],
                    pa[:, g * SPAN * F:(g + 1) * SPAN * F],
                    mybir.ActivationFunctionType.Copy,
                    bias=0.0,
                    scale=1.0,
                    accum_out=racc[:, g:g + 1],
                )
            nc.vector.tensor_add(master4[:], master4[:], racc[:])

            res = master_pool.tile([128, 9], fp32, tag="res")
            nc.vector.tensor_copy(res[:, 0:G_ACT], master4[:])
            for gi in range(G_PE):
                nc.vector.tensor_reduce(
                    res[:, G_ACT + gi:G_ACT + gi + 1],
                    accP[gi][:],
                    axis=mybir.AxisListType.X,
                    op=mybir.AluOpType.add,
                )
            nc.vector.tensor_copy(res[:, 8:9], master_se[:])
            nc.sync.dma_start(out_dram[:], res[:])

    nc.compile()
    return nc


def _pack_core(e_core):
    """[N_PER_CORE, 64] fp32 -> [128, COLS] fp16 in the (j, e, g) layout."""
    a = e_core.reshape(N_CHUNKS, F, J, G, 8)        # [ch, fl, j, g, e]
    a = a.transpose(0, 2, 4, 3, 1)                  # [ch, j, e, g, fl]
    a = a.reshape(N_CHUNKS, 128, CH_COLS)
    a = a.transpose(1, 0, 2).reshape(128, COLS)
    return np.ascontiguousarray(a).astype(np.float16)


def _make_consts(query):
    c_shift = float(6.0 * np.linalg.norm(query))
    qmats = np.zeros((128, 128 * G), dtype=np.float32)
    for g in range(G):
        for j in range(J):
            qmats[8 * j:8 * j + 8, g * 128 + 8 * j:g * 128 + 8 * j + 8] = (
                query[8 * g:8 * g + 8][:, None]
            )
    ident = np.eye(128, dtype=np.float32)
    cshift = np.full((128, 1), -c_shift, dtype=np.float32)
    return (
        qmats.astype(np.float16),
        ident.astype(np.float16),
        cshift,
    )


def build_in_maps(embeddings, query):
    embeddings = np.asarray(embeddings, dtype=np.float32)
    query = np.asarray(query, dtype=np.float32)
    qmats, ident, cshift = _make_consts(query)
    in_maps = []
    for c in range(N_CORES):
        e_core = embeddings[c * N_PER_CORE:(c + 1) * N_PER_CORE]
        in_maps.append({
            "epack": _pack_core(e_core),
            "qmats": qmats,
            "ident": ident,
            "cshift": cshift,
        })
    return in_maps


def combine_results(results):
    num = np.zeros(D, dtype=np.float64)
    z = 0.0
    for r in results:
        o = r["out"].astype(np.float64)              # [128, 9]
        for g in range(G):
            num[8 * g:8 * g + 8] += o[:, g].reshape(J, 8).sum(axis=0)
        z += o[0:128:8, 8].sum()
    return (num / z).astype(np.float32)


def get_nc():
    if "nc" not in _compiled:
        _compiled["nc"] = _build_nc()
    return _compiled["nc"]


def kernel(embeddings, query):
    from concourse.bass_utils import run_bass_kernel_spmd

    nc = get_nc()
    in_maps = build_in_maps(embeddings, query)

    res = None
    for attempt in range(3):
        try:
            res = run_bass_kernel_spmd(nc, in_maps, list(range(N_CORES)))
            break
        except Exception:
            if attempt == 2:
                raise

    return combine_results(res.results)


# revision 8
# speedup vs baseline: 1.6942x; 1.2424x over previous
"""Attention pooling kernel for Trainium2 (8 NeuronCores) — fp16 pipeline.

Computes: scores = E @ q; w = softmax(scores); out = w @ E
for E [N=2097152, 64] fp32, q [64] fp32.

Strategy (per core, N/8 = 262144 rows), all heavy traffic in fp16:
  - Host casts the core's row-shard to fp16 and packs it as
    ep [128, 131072]: with n = 16*f + j (j in [0,16)) and d = 8*g + e
    (e in [0,8)), partition k = 8*j + e holds E[16f+j, 8g+e] at column
    ch*8192 + g*1024 + fl  (f = ch*1024 + fl; 16 chunks of 8192 cols).
  - Scores: 8 chained matmuls per 512-col half (stationary Qg [128,128],
    Qg[8j+e, 8j'+e'] = q[8g+e]*(j==j')) accumulate in PSUM ->
    scores s(16f+j) land replicated over the 8 e'-slots of parity j.
    exp therefore only touches [128, F] per chunk — 8x less ACT work
    than a 2-parity layout.
  - ACT: w = exp(scores - C) -> fp16, fused accum_out gives sumexp
    partials (C computed from q alone; cancels in the final division).
  - DVE: per chunk 8 tensor_tensor fp16 multiplies (2x perf mode):
    prod_g = ep_g * w.  The f-reduction of prod_g is split:
      g=0..3 -> ACT Copy with accum_out over 2-chunk spans
      g=4..7 -> PE identity-stationary matmuls (fp8 identity: cheap
                LDWEIGHTS) accumulating into 4 persistent PSUM banks,
                emitted one chunk late so PE never stalls on the
                exp->mult chain.
    The last span's ACT groups instead use fused DVE
    scalar_tensor_tensor (mult+accum) to shorten the drain tail.
  - Host: out[d] = sum_cores sum_j acc[(j,e),g] / sum exp, d = 8g+e.
"""

import sys

sys.path.insert(0, "/opt/trn_rl_repo")

import numpy as np

N_TOTAL = 2097152
D = 64
N_CORES = 8
N_PER_CORE = N_TOTAL // N_CORES          # 262144
J = 16                                   # parities (n mod 16)
G = 8                                    # dim groups of 8
F_TOT = N_PER_CORE // J                  # 16384 f-columns per core
F = 1024                                 # f-columns per chunk
N_CHUNKS = F_TOT // F                    # 16
CH_COLS = G * F                          # 8192 packed cols per chunk
COLS = F_TOT * G                         # 131072 packed cols per core
SPAN = 2                                 # chunks per ACT reduce span
G_ACT = 4                                # groups 0..3 reduced on ACT
MM_N = 512                               # matmul free dim (one PSUM bank)

_compiled = {}


def _build_nc():
    import concourse.bacc as bacc
    import concourse.bass as bass
    import concourse.mybir as mybir
    import concourse.tile as tile

    fp32 = mybir.dt.float32
    f16 = mybir.dt.float16
    f8 = mybir.dt.float8e4

    nc = bacc.Bacc()
    ep_dram = nc.declare_dram_parameter("epack", [128, COLS], f16, isOutput=False)
    qm_dram = nc.declare_dram_parameter("qmats", [128, 128 * G], f16, isOutput=False)
    id_dram = nc.declare_dram_parameter("ident", [128, 128], f8, isOutput=False)
    cs_dram = nc.declare_dram_parameter("cshift", [128, 1], fp32, isOutput=False)
    out_dram = nc.declare_dram_parameter("out", [128, 9], fp32, isOutput=True)

    n_halves = F // MM_N
    G_PE = G - G_ACT
    LAST_SPAN_T0 = N_CHUNKS - SPAN       # chunks >= this use the stt tail

    with tile.TileContext(nc) as tc:
        with (
            tc.tile_pool(name="const", bufs=1) as const_pool,
            tc.tile_pool(name="ep", bufs=5) as ep_pool,
            tc.tile_pool(name="w", bufs=3) as w_pool,
            tc.tile_pool(name="prodA", bufs=3) as prodA_pool,
            tc.tile_pool(name="prodS", bufs=2) as prodS_pool,
            tc.tile_pool(name="junk", bufs=1) as junk_pool,
            tc.tile_pool(name="sm", bufs=3) as sm_pool,
            tc.tile_pool(name="racc", bufs=2) as racc_pool,
            tc.tile_pool(name="master", bufs=1) as master_pool,
            tc.tile_pool(name="ps", bufs=2, space=bass.MemorySpace.PSUM) as ps_pool,
            tc.tile_pool(name="acc", bufs=1, space=bass.MemorySpace.PSUM) as acc_pool,
        ):
            qmats = const_pool.tile([128, 128 * G], f16, tag="qmats")
            ident = const_pool.tile([128, 128], f8, tag="ident")
            cshift = const_pool.tile([128, 1], fp32, tag="cshift")
            nc.sync.dma_start(qmats[:], qm_dram[:])
            nc.sync.dma_start(ident[:], id_dram[:])
            nc.sync.dma_start(cshift[:], cs_dram[:])

            master4 = master_pool.tile([128, G_ACT], fp32, tag="master4")
            master_se = master_pool.tile([128, 1], fp32, tag="master_se")
            stt_acc = master_pool.tile([128, SPAN * G_ACT], fp32, tag="stt_acc")
            accP = [
                acc_pool.tile([128, MM_N], fp32, tag=f"accP{gi}", name=f"accP{gi}")
                for gi in range(G_PE)
            ]

            pend = None          # (t, [prodS tiles]) awaiting PE reduce
            pend_act = None      # (prodA tile, racc tile) awaiting ACT reduce
            prodA = None

            def emit_pe_reduce(prev_t, tiles):
                for gi, pt in enumerate(tiles):
                    for h in range(n_halves):
                        nc.tensor.matmul(
                            accP[gi][:],
                            ident[:],
                            pt[:, h * MM_N:(h + 1) * MM_N],
                            start=(prev_t == 0 and h == 0),
                            stop=(prev_t == N_CHUNKS - 1 and h == n_halves - 1),
                        )

            for t in range(N_CHUNKS):
                ep = ep_pool.tile([128, CH_COLS], f16, tag="ep")
                if t == 0:
                    # per-slab DMAs so the first score matmuls start early
                    for g in range(G):
                        nc.sync.dma_start(
                            ep[:, g * F:(g + 1) * F],
                            ep_dram[:, g * F:(g + 1) * F],
                        )
                else:
                    nc.sync.dma_start(
                        ep[:], ep_dram[:, t * CH_COLS:(t + 1) * CH_COLS]
                    )

                ps = ps_pool.tile([128, F], fp32, tag="ps")
                for h in range(n_halves):
                    lo = h * MM_N
                    for g in range(G):
                        nc.tensor.matmul(
                            ps[:, lo:lo + MM_N],
                            qmats[:, g * 128:(g + 1) * 128],
                            ep[:, g * F + lo:g * F + lo + MM_N],
                            start=(g == 0),
                            stop=(g == G - 1),
                        )
                # PE reduce of the previous chunk's prod tiles (one chunk
                # late so PE doesn't stall on the exp->mult chain)
                if pend is not None:
                    emit_pe_reduce(*pend)

                w = w_pool.tile([128, F], f16, tag="w")
                se = sm_pool.tile([128, 1], fp32, tag="se")
                nc.scalar.activation(
                    w[:],
                    ps[:],
                    mybir.ActivationFunctionType.Exp,
                    bias=cshift[:, 0:1],
                    scale=1.0,
                    accum_out=se[:],
                )
                # two ACT span-reduce ops per chunk, for the previous span
                if pend_act is not None:
                    pa, racc = pend_act
                    for g in (0, 1) if t % SPAN == 0 else (2, 3):
                        junk = junk_pool.tile([128, SPAN * F], f16, tag="junk")
                        nc.scalar.activation(
                            junk[:],
                            pa[:, g * SPAN * F:(g + 1) * SPAN * F],
                            mybir.ActivationFunctionType.Copy,
                            bias=0.0,
                            scale=1.0,
                            accum_out=racc[:, g:g + 1],
                        )
                    if t % SPAN == SPAN - 1:
                        if t == SPAN + SPAN - 1:
                            nc.vector.tensor_copy(master4[:], racc[:])
                        else:
                            nc.vector.tensor_add(master4[:], master4[:], racc[:])
                        pend_act = None

                if t == 0:
                    nc.vector.tensor_copy(master_se[:], se[:])
                else:
                    nc.vector.tensor_add(master_se[:], master_se[:], se[:])

                # PE-path mults first: the next chunk's PE reduce waits on them
                cur = []
                for g in range(G_ACT, G):
                    pS = prodS_pool.tile([128, F], f16, tag=f"prodS{g}")
                    nc.vector.tensor_mul(pS[:], ep[:, g * F:(g + 1) * F], w[:])
                    cur.append(pS)
                pend = (t, cur)

                if t < LAST_SPAN_T0:
                    if t % SPAN == 0:
                        prodA = prodA_pool.tile(
                            [128, G_ACT * SPAN * F], f16, tag="prodA"
                        )
                    for g in range(G_ACT):
                        dst = prodA[
                            :,
                            (g * SPAN + (t % SPAN)) * F:(g * SPAN + (t % SPAN) + 1) * F,
                        ]
                        nc.vector.tensor_mul(dst, ep[:, g * F:(g + 1) * F], w[:])
                    if t % SPAN == SPAN - 1:
                        racc = racc_pool.tile([128, G_ACT], fp32, tag="racc")
                        pend_act = (prodA, racc)
                else:
                    # last span: fused mult+accum on DVE, no ACT reduce tail
                    for g in range(G_ACT):
                        junk = junk_pool.tile([128, F], f16, tag="sjunk")
                        nc.vector.scalar_tensor_tensor(
                            junk[:],
                            ep[:, g * F:(g + 1) * F],
                            1.0,
                            w[:],
                            op0=mybir.AluOpType.mult,
                            op1=mybir.AluOpType.mult,
                            accum_out=stt_acc[:, (t - LAST_SPAN_T0) * G_ACT + g:
                                              (t - LAST_SPAN_T0) * G_ACT + g + 1],
                        )

            # drain: last chunk's PE reduce, stt-tail accums, final combine
            emit_pe_reduce(*pend)
            assert pend_act is None
            for sl in range(SPAN):
                nc.vector.tensor_add(
                    master4[:], master4[:],
                    stt_acc[:, sl * G_ACT:(sl + 1) * G_ACT],
                )

            res = master_pool.tile([128, 9], fp32, tag="res")
            nc.vector.tensor_copy(res[:, 0:G_ACT], master4[:])
            for gi in range(G_PE):
                nc.vector.tensor_reduce(
                    res[:, G_ACT + gi:G_ACT + gi + 1],
                    accP[gi][:],
                    axis=mybir.AxisListType.X,
                    op=mybir.AluOpType.add,
                )
            nc.vector.tensor_copy(res[:, 8:9], master_se[:])
            nc.sync.dma_start(out_dram[:], res[:])

    nc.compile()
    return nc


def _pack_core(e_core):
    """[N_PER_CORE, 64] fp32 -> [128, COLS] fp16 in the (j, e, g) layout."""
    a = e_core.reshape(N_CHUNKS, F, J, G, 8)        # [ch, fl, j, g, e]
    a = a.transpose(0, 2, 4, 3, 1)                  # [ch, j, e, g, fl]
    a = a.reshape(N_CHUNKS, 128, CH_COLS)
    a = a.transpose(1, 0, 2).reshape(128, COLS)
    return np.ascontiguousarray(a).astype(np.float16)


def _make_consts(query):
    import ml_dtypes

    c_shift = float(6.0 * np.linalg.norm(query))
    qmats = np.zeros((128, 128 * G), dtype=np.float32)
    for g in range(G):
        for j in range(J):
            qmats[8 * j:8 * j + 8, g * 128 + 8 * j:g * 128 + 8 * j + 8] = (
                query[8 * g:8 * g + 8][:, None]
            )
    ident = np.eye(128, dtype=np.float32)
    cshift = np.full((128, 1), -c_shift, dtype=np.float32)
    return (
        qmats.astype(np.float16),
        ident.astype(ml_dtypes.float8_e4m3),
        cshift,
    )


def build_in_maps(embeddings, query):
    embeddings = np.asarray(embeddings, dtype=np.float32)
    query = np.asarray(query, dtype=np.float32)
    qmats, ident, cshift = _make_consts(query)
    in_maps = []
    for c in range(N_CORES):
        e_core = embeddings[c * N_PER_CORE:(c + 1) * N_PER_CORE]
        in_maps.append({
            "epack": _pack_core(e_core),
            "qmats": qmats,
            "ident": ident,
            "cshift": cshift,
        })
    return in_maps


def combine_results(results):
    num = np.zeros(D, dtype=np.float64)
    z = 0.0
    for r in results:
        o = r["out"].astype(np.float64)              # [128, 9]
        for g in range(G):
            num[8 * g:8 * g + 8] += o[:, g].reshape(J, 8).sum(axis=0)
        z += o[0:128:8, 8].sum()
    return (num / z).astype(np.float32)


def get_nc():
    if "nc" not in _compiled:
        _compiled["nc"] = _build_nc()
    return _compiled["nc"]


def kernel(embeddings, query):
    from concourse.bass_utils import run_bass_kernel_spmd

    nc = get_nc()
    in_maps = build_in_maps(embeddings, query)

    res = None
    for attempt in range(3):
        try:
            res = run_bass_kernel_spmd(nc, in_maps, list(range(N_CORES)))
            break
        except Exception:
            if attempt == 2:
                raise

    return combine_results(res.results)
